# revision 65
# baseline (speedup 1.0000x reference)
# Multi-head attention kernel for Trainium2, sharded over 8 NeuronCores.
#
# Sharding: core = (batch b, query-chunk qc). Each core handles QB=512 queries
# of one batch, all 12 heads, recomputing the K/V projections for its batch.
# (Cross-core dedup was measured and rejected: AllGather of the K/V quarters
# has a ~45-105us ncfw control-plane floor in this environment, far too slow
# to land before attention needs the remote key tiles.)
#
# Layout strategy (bf16 matmul operands, fp32 PSUM accumulation/epilogues):
#   - Host pre-transposes activations to [E, S] so the contraction dim (E)
#     lands on SBUF partitions; fp32 matmul is avoided on-device (it lowers to
#     two PE passes), so all matmul operands are bf16.
#   - q^T, k^T computed as [768, S] via lhsT=W chunks; per-partition bias
#     added during the PSUM->SBUF copy on ScalarE (idle pre-attention).
#   - k^T projection is woven with head-pair-0's scores+exp per 512-key block
#     so ScalarE's exp stream starts ~18us in, not after all projections.
#   - v computed directly as [keys, 768] in two independent 384-wide psum
#     chains (heads 0-5 / 6-11) so the per-tile copy never serializes the
#     pipeline; stored with a ones-column per head ([128,16,12,65]) so the PV
#     matmul (M=65) also produces the softmax denominator row for free.
#     hp0's PV drains inside this loop, releasing exp tiles early.
#   - scores^T = [keys, queries] per head: K=64 matmuls; even/odd heads sit in
#     partition halves 0-63/64-127, emitted adjacently so they land in
#     disjoint PE row groups and run concurrently (row packing).
#   - exp on ScalarE in [128, 2x512] groups PSUM->SBUF (bf16), streamed
#     straight into the accumulating PV matmul (no full score matrix in SBUF);
#     epool holds 20 tiles so exp can run a full head-pair ahead of PV.
#   - softmax normalize is streamed per head-pair right after its PV finishes
#     (overlaps the next pair's attention): stage o_raw pair-stacked in
#     partition halves (fast PSUM release), copy the denominator rows to SBUF,
#     one fast-approx DVE reciprocal + bf16 cast, K=1 broadcast matmuls into
#     psB-bank bc halves (so the PV accumulators in psA are never blocked),
#     DVE multiplies. Note: custom-DVE ops (reciprocal_approx_*) must NOT
#     read PSUM directly (raw accumulator bits, garbage results); DVE ops
#     need 32-aligned base partitions; matmul accumulation groups must not
#     mix tile positions.
#   - output projection computes y^T = Wo^T o: per e-chunk, 6 K=128 head-pair
#     contractions, all N=512, alternating psA/psB banks so two chains
#     pipeline; bias (bv@Wo + bo, per-partition on e_out) added on the
#     PSUM->SBUF copy; host transposes y^T back in assemble().

import numpy as np
from contextlib import ExitStack

import concourse.bass as bass
import concourse.mybir as mybir
import concourse.tile as tile
from concourse import bacc
from concourse.bass_utils import run_bass_kernel_spmd

F32 = mybir.dt.float32
BF16 = mybir.dt.bfloat16
P = 128
E = 768
S = 2048
B = 2
H = 12
D = 64
QB = 512          # queries per core
NCORES = 8
EC = E // P       # 6 e-chunks
KT = S // P       # 16 key tiles
MT_Q = E // P     # 6 M-tiles for q^T/k^T (768 rows)
NC4 = S // 512    # 4 n-slices of k^T


def build_nc():
    nc = bacc.Bacc("TRN2", debug=False)

    # DRAM I/O (per-core shapes; same NEFF on all 8 cores)
    xq = nc.dram_tensor("xq", (E, QB), BF16, kind="ExternalInput")     # query[b,chunk].T
    xk = nc.dram_tensor("xk", (E, S), BF16, kind="ExternalInput")      # key[b].T
    xv = nc.dram_tensor("xv", (E, S), BF16, kind="ExternalInput")      # value[b].T
    wq = nc.dram_tensor("wq", (E, E), BF16, kind="ExternalInput")      # [E, H*D], pre-scaled 1/sqrt(D)
    wk = nc.dram_tensor("wk", (E, E), BF16, kind="ExternalInput")
    wv = nc.dram_tensor("wv", (E, E), BF16, kind="ExternalInput")
    wo = nc.dram_tensor("wo", (E, E), BF16, kind="ExternalInput")
    bq = nc.dram_tensor("bq", (P, MT_Q), F32, kind="ExternalInput")   # per-partition bias per M-tile
    bk = nc.dram_tensor("bk", (P, MT_Q), F32, kind="ExternalInput")
    bo2 = nc.dram_tensor("bo2", (P, EC), F32, kind="ExternalInput")   # (bv@Wo + bo) per e_out partition
    sel2 = nc.dram_tensor("sel2", (1, D), BF16, kind="ExternalInput")  # ones row for K=1 broadcast
    out = nc.dram_tensor("out", (E, QB), F32, kind="ExternalOutput")  # y^T

    with tile.TileContext(nc) as tc:
        with ExitStack() as ctx:
            _emit(ctx, tc, nc, xq, xk, xv, wq, wk, wv, wo, bq, bk, bo2, sel2, out)
    nc.compile()
    return nc


def _emit(ctx, tc, nc, xq, xk, xv, wq, wk, wv, wo, bq, bk, bo2, sel2, out):
    # ---- pools ----
    persist = ctx.enter_context(tc.tile_pool(name="persist", bufs=1))
    # big weight slots [128, 6, 768] reused wq -> wk -> wv -> wo
    wpool = ctx.enter_context(tc.tile_pool(name="wpool", bufs=2))
    xpool = ctx.enter_context(tc.tile_pool(name="xpool", bufs=2))
    xvpool = ctx.enter_context(tc.tile_pool(name="xvpool", bufs=3))
    epool = ctx.enter_context(tc.tile_pool(name="epool", bufs=20))
    dpool = ctx.enter_context(tc.tile_pool(name="dpool", bufs=2))
    outpool = ctx.enter_context(tc.tile_pool(name="outpool", bufs=6))
    # PSUM budget 8 banks/partition: psA 2 + psB 2 + psC 4
    psA = ctx.enter_context(tc.tile_pool(name="psA", bufs=2, space="PSUM"))   # [128,512] proj qk + PV out + O-proj
    psB = ctx.enter_context(tc.tile_pool(name="psB", bufs=2, space="PSUM"))   # [128,512] v chains / bc / O-proj
    psC = ctx.enter_context(tc.tile_pool(name="psC", bufs=2, space="PSUM"))   # [128,2,512] scores

    # ---- persistent SBUF tensors ----
    qT = persist.tile([P, MT_Q, QB], BF16)       # q^T [768, QB]
    kT = persist.tile([P, MT_Q, S], BF16)        # k^T [768, S]
    v_sb = persist.tile([P, KT, H, D + 1], BF16)  # v + ones column per head
    o_all = persist.tile([P, H // 2, QB], BF16)   # normalized o^T, head pairs in partition halves
    o_raw = persist.tile([P, H // 2, QB], F32)    # unnormalized o^T, pair-stacked
    bq_sb = persist.tile([P, MT_Q], F32)
    bk_sb = persist.tile([P, MT_Q], F32)
    bo2_sb = persist.tile([P, EC], F32)
    sel_sb = persist.tile([1, D], BF16)

    # first-needed DMAs first; constants go on the scalar HWDGE queue
    wq_t = wpool.tile([P, EC, E], BF16, tag="w18")
    xq_t = xpool.tile([P, EC, QB], BF16, tag="xs")
    qs_ = [nc.sync, nc.gpsimd, nc.scalar]
    for ec in range(EC):
        qs_[ec % 3].dma_start(wq_t[:, ec, :], wq[ec * P:(ec + 1) * P, :])
        qs_[(ec + 1) % 3].dma_start(xq_t[:, ec, :], xq[ec * P:(ec + 1) * P, :])
    nc.scalar.dma_start(bq_sb[:], bq[:])
    nc.scalar.dma_start(bk_sb[:], bk[:])
    nc.scalar.dma_start(bo2_sb[:], bo2[:])
    nc.scalar.dma_start(sel_sb[:], sel2[:])

    # ones columns for denominator (written once; v-proj copies don't touch col D)
    nc.vector.memset(v_sb[:, :, :, D], 1.0)

    # ---- q^T projection ----
    for mt in range(MT_Q):
        ps = psA.tile([P, 512], F32, tag="psA")
        for ec in range(EC):
            nc.tensor.matmul(ps[:], wq_t[:, ec, mt * P:(mt + 1) * P], xq_t[:, ec, :],
                             start=(ec == 0), stop=(ec == EC - 1))
        nc.scalar.add(qT[:, mt, :], ps[:], bq_sb[:, mt:mt + 1])

    # ---- k^T projection, woven with hp0's scores+exp so ScalarE starts early ----
    def scores_exp(hp, kt):
        st = psC.tile([P, 2, 512], F32, tag="psC")
        for i in range(2):
            po = D * i      # partition offset of this head's d-rows
            nc.tensor.matmul(st[:, i, :],
                             kT[po:po + D, hp, kt * P:(kt + 1) * P],
                             qT[po:po + D, hp, :],
                             start=True, stop=True)
        ex = epool.tile([P, 2, 512], BF16, tag="ex", name=f"ex{hp}_{kt}")
        nc.scalar.activation(ex[:, :, :], st[:, :, :], mybir.ActivationFunctionType.Exp)
        return ex

    ex0 = {}
    wk_t = wpool.tile([P, EC, E], BF16, tag="w18")
    for ec in range(EC):
        qs_[ec % 3].dma_start(wk_t[:, ec, :], wk[ec * P:(ec + 1) * P, :])
    def v_proj(kt):
        # two independent 384-wide chains (heads 0-5 / 6-11) so the per-kt
        # psum->sbuf copy doesn't serialize the whole v pipeline
        xv_t = xvpool.tile([P, EC, P], BF16, tag="xv")
        nc.gpsimd.dma_start(
            xv_t[:], xv[:, kt * P:(kt + 1) * P].rearrange("(ec p) s -> p ec s", p=P))
        for half in range(2):
            # [128,512] slot (O-proj reuses these banks later); v uses 384 cols
            psv = psB.tile([P, 512], F32, tag="psB", name=f"psv{half}")
            for ec in range(EC):
                nc.tensor.matmul(psv[:, 0:384], xv_t[:, ec, :], wv_t[:, ec, 384 * half:384 * (half + 1)],
                                 start=(ec == 0), stop=(ec == EC - 1))
            # strided copy into per-head slots (leaves ones column intact)
            nc.vector.tensor_copy(v_sb[:, kt, 6 * half:6 * (half + 1), 0:D],
                                  psv[:, 0:384].rearrange("p (h d) -> p h d", d=D))

    for n4 in range(NC4):
        xk_t = xpool.tile([P, EC, 512], BF16, tag="xs")
        nc.sync.dma_start(xk_t[:], xk[:, n4 * 512:(n4 + 1) * 512].rearrange("(ec p) s -> p ec s", p=P))
        for mt in range(MT_Q):
            ps = psA.tile([P, 512], F32, tag="psA")
            for ec in range(EC):
                nc.tensor.matmul(ps[:], wk_t[:, ec, mt * P:(mt + 1) * P], xk_t[:, ec, :],
                                 start=(ec == 0), stop=(ec == EC - 1))
            nc.scalar.add(kT[:, mt, n4 * 512:(n4 + 1) * 512], ps[:], bk_sb[:, mt:mt + 1])
        for kt in range(4 * n4, 4 * n4 + 4):
            ex0[kt] = scores_exp(0, kt)

    # ---- attention: head pairs, normalize streamed per pair ----
    # Per key tile: both heads' score matmuls are adjacent K=64 ops on
    # disjoint PE row groups (partitions 0-63 / 64-127) -> run concurrently.
    def pv(hp, kt, o_ps, ex):
        for i in range(2):
            nc.tensor.matmul(o_ps[i][0:D + 1, :],
                             v_sb[:, kt, 2 * hp + i, :],
                             ex[:, i, :],
                             start=(kt == 0), stop=(kt == KT - 1))

    wv_t = wpool.tile([P, EC, E], BF16, tag="w18")
    for ec in range(EC):
        nc.sync.dma_start(wv_t[:, ec, :], wv[ec * P:(ec + 1) * P, :])
    # hp0's PV drains inside the v loop: k-proj's psA traffic is done, so the
    # pair of accumulators can stay live across it (frees ex tiles early)
    o_ps0 = {i: psA.tile([P, 512], F32, tag="psA", name=f"o_ps{i}") for i in range(2)}
    for kt in range(KT):
        v_proj(kt)
        pv(0, kt, o_ps0, ex0[kt])

    # prefetch wo for the output projection (slot reuse after wq)
    wo_t = wpool.tile([P, EC, E], BF16, tag="w18")
    nc.sync.dma_start(wo_t[:], wo[:].rearrange("(ec p) m -> p ec m", p=P))

    def normalize(hp, o_ps):
        # stage pair-stacked o_raw (frees PSUM), reciprocal of the denominator
        # rows (fast approx via SBUF bounce), bf16 cast, K=1 broadcast matmuls
        # into psB-bank bc halves, DVE multiplies.
        dens_t = dpool.tile([1, 2, 512], F32, tag="dens", name="dens")
        drec_t = dpool.tile([1, 2, 512], F32, tag="drec", name="drec")
        drec_b = dpool.tile([1, 2, 512], BF16, tag="drecb", name="drecb")
        for i in range(2):
            nc.vector.tensor_copy(o_raw[D * i:D * i + D, hp, :], o_ps[i][0:D, :])
            nc.vector.tensor_copy(dens_t[0:1, i, :], o_ps[i][D:D + 1, :])
        nc.vector.reciprocal_approx_fast(drec_t[:], dens_t[:])
        nc.vector.tensor_copy(drec_b[:], drec_t[:])
        bc_ps = psB.tile([P, 512], F32, tag="psB", name="bc")
        for i in range(2):
            nc.tensor.matmul(bc_ps[D * i:D * i + D, :], sel_sb[0:1, 0:D],
                             drec_b[0:1, i, :], start=True, stop=True)
        nc.vector.tensor_tensor(o_all[:, hp, :], o_raw[:, hp, :], bc_ps[:],
                                mybir.AluOpType.mult)

    normalize(0, o_ps0)
    for hp in range(1, H // 2):
        o_ps = {i: psA.tile([P, 512], F32, tag="psA", name=f"o_ps{i}") for i in range(2)}
        for kt in range(KT):
            ex = scores_exp(hp, kt)
            pv(hp, kt, o_ps, ex)
        normalize(hp, o_ps)

    # ---- output projection: y^T[e_out, q] per e-chunk, K=128 per head pair ----
    # hp5 contracts last so hp0-4 matmuls overlap hp5's normalize chain;
    # bf16 output halves the final write-out DMA (error ~0.2% << gate)
    # alternate psA/psB banks so two e-chunk chains pipeline concurrently
    for ec in range(EC):
        pool, tag = (psA, "psA") if ec % 2 == 0 else (psB, "psB")
        po = pool.tile([P, 512], F32, tag=tag, name="psO")
        for hp in range(H // 2):
            nc.tensor.matmul(po[:], wo_t[:, hp, ec * P:(ec + 1) * P], o_all[:, hp, :],
                             start=(hp == 0), stop=(hp == H // 2 - 1))
        out_sb = outpool.tile([P, 512], F32, tag="outsb")
        nc.vector.tensor_scalar_add(out_sb[:], po[:], bo2_sb[:, ec:ec + 1])
        qs_[ec % 3].dma_start(out[ec * P:(ec + 1) * P, :], out_sb[:])


_NC_CACHE = None


def _get_nc():
    global _NC_CACHE
    if _NC_CACHE is None:
        _NC_CACHE = build_nc()
    return _NC_CACHE


def make_in_maps(query, key_, value, Wq, bq, Wk, bk, Wv, bv, Wo, bo):
    """Host-side sharding + layout prep. Returns list of 8 input dicts."""
    query = np.asarray(query, dtype=np.float32)
    key_ = np.asarray(key_, dtype=np.float32)
    value = np.asarray(value, dtype=np.float32)
    scale = 1.0 / np.sqrt(np.float32(D))

    import ml_dtypes
    BF = ml_dtypes.bfloat16
    wq_f = (np.ascontiguousarray(np.transpose(np.asarray(Wq, np.float32), (1, 0, 2)).reshape(E, E)) * scale).astype(BF)
    wk_f = np.ascontiguousarray(np.transpose(np.asarray(Wk, np.float32), (1, 0, 2)).reshape(E, E)).astype(BF)
    wv_f = np.ascontiguousarray(np.transpose(np.asarray(Wv, np.float32), (1, 0, 2)).reshape(E, E)).astype(BF)
    wo_f = np.ascontiguousarray(np.asarray(Wo, np.float32)).astype(BF)

    bq_f = (np.asarray(bq, np.float32).reshape(E) * scale).reshape(MT_Q, P).T.copy()
    bk_f = np.asarray(bk, np.float32).reshape(E).reshape(MT_Q, P).T.copy()
    bv_f = np.asarray(bv, np.float32).reshape(E)
    wo_f32 = wo_f.astype(np.float32)
    bo2_f = (bv_f @ wo_f32 + np.asarray(bo, np.float32)).reshape(EC, P).T.copy()

    xk_t = [np.ascontiguousarray(key_[b].T).astype(BF) for b in range(B)]
    xv_t = [np.ascontiguousarray(value[b].T).astype(BF) for b in range(B)]

    sel_np = np.ones((1, D), dtype=np.float32).astype(BF)

    in_maps = []
    for core in range(NCORES):
        b = core // (NCORES // B)
        qc = core % (NCORES // B)
        xq_t = np.ascontiguousarray(query[b, qc * QB:(qc + 1) * QB, :].T).astype(BF)
        in_maps.append({
            "xq": xq_t, "xk": xk_t[b], "xv": xv_t[b],
            "wq": wq_f, "wk": wk_f, "wv": wv_f, "wo": wo_f,
            "bq": bq_f, "bk": bk_f, "bo2": bo2_f, "sel2": sel_np,
        })
    return in_maps


def assemble(results):
    outp = np.empty((B, S, E), dtype=np.float32)
    for core in range(NCORES):
        b = core // (NCORES // B)
        qc = core % (NCORES // B)
        outp[b, qc * QB:(qc + 1) * QB, :] = results[core]["out"].T.astype(np.float32)
    return outp


def kernel(query, key_, value, Wq, bq, Wk, bk, Wv, bv, Wo, bo):
    nc = _get_nc()
    in_maps = make_in_maps(query, key_, value, Wq, bq, Wk, bk, Wv, bv, Wo, bo)
    res = run_bass_kernel_spmd(nc, in_maps, core_ids=list(range(NCORES)))
    return assemble(res.results)


# revision 66
# speedup vs baseline: 1.0221x; 1.0221x over previous
# Multi-head attention kernel for Trainium2, sharded over 8 NeuronCores.
#
# Sharding: core = (batch b, query-chunk qc). Each core handles QB=512 queries
# of one batch, all 12 heads, recomputing the K/V projections for its batch.
# (Cross-core dedup was measured and rejected: AllGather of the K/V quarters
# has a ~45-105us ncfw control-plane floor in this environment, far too slow
# to land before attention needs the remote key tiles.)
#
# Layout strategy (bf16 matmul operands, fp32 PSUM accumulation/epilogues):
#   - Host pre-transposes activations to [E, S] so the contraction dim (E)
#     lands on SBUF partitions; fp32 matmul is avoided on-device (it lowers to
#     two PE passes), so all matmul operands are bf16.
#   - q^T, k^T computed as [768, S] via lhsT=W chunks; per-partition bias
#     added during the PSUM->SBUF copy on ScalarE (idle pre-attention).
#   - k^T projection is woven with head-pair-0's scores+exp per 512-key block
#     so ScalarE's exp stream starts ~18us in, not after all projections.
#   - v computed directly as [keys, 768] in two independent 384-wide psum
#     chains (heads 0-5 / 6-11) so the per-tile copy never serializes the
#     pipeline; stored with a ones-column per head ([128,16,12,65]) so the PV
#     matmul (M=65) also produces the softmax denominator row for free.
#     hp0's PV drains inside this loop, releasing exp tiles early.
#   - scores^T = [keys, queries] per head: K=64 matmuls; even/odd heads sit in
#     partition halves 0-63/64-127, emitted adjacently so they land in
#     disjoint PE row groups and run concurrently (row packing).
#   - exp on ScalarE in [128, 2x512] groups PSUM->SBUF (bf16), streamed
#     straight into the accumulating PV matmul (no full score matrix in SBUF);
#     epool holds 20 tiles so exp can run a full head-pair ahead of PV.
#   - softmax normalize is streamed per head-pair right after its PV finishes
#     (overlaps the next pair's attention): stage o_raw pair-stacked in
#     partition halves (fast PSUM release), copy the denominator rows to SBUF,
#     one fast-approx DVE reciprocal + bf16 cast, K=1 broadcast matmuls into
#     psB-bank bc halves (so the PV accumulators in psA are never blocked),
#     DVE multiplies. Note: custom-DVE ops (reciprocal_approx_*) must NOT
#     read PSUM directly (raw accumulator bits, garbage results); DVE ops
#     need 32-aligned base partitions; matmul accumulation groups must not
#     mix tile positions.
#   - output projection computes y^T = Wo^T o: per e-chunk, 6 K=128 head-pair
#     contractions, all N=512, alternating psA/psB banks so two chains
#     pipeline; bias (bv@Wo + bo, per-partition on e_out) added on the
#     PSUM->SBUF copy; host transposes y^T back in assemble().

import numpy as np
from contextlib import ExitStack

import concourse.bass as bass
import concourse.mybir as mybir
import concourse.tile as tile
from concourse import bacc
from concourse.bass_utils import run_bass_kernel_spmd

F32 = mybir.dt.float32
BF16 = mybir.dt.bfloat16
P = 128
E = 768
S = 2048
B = 2
H = 12
D = 64
QB = 512          # queries per core
NCORES = 8
EC = E // P       # 6 e-chunks
KT = S // P       # 16 key tiles
MT_Q = E // P     # 6 M-tiles for q^T/k^T (768 rows)
NC4 = S // 512    # 4 n-slices of k^T


def build_nc():
    nc = bacc.Bacc("TRN2", debug=False)

    # DRAM I/O (per-core shapes; same NEFF on all 8 cores)
    xq = nc.dram_tensor("xq", (E, QB), BF16, kind="ExternalInput")     # query[b,chunk].T
    xk = nc.dram_tensor("xk", (E, S), BF16, kind="ExternalInput")      # key[b].T
    xv = nc.dram_tensor("xv", (E, S), BF16, kind="ExternalInput")      # value[b].T
    wq = nc.dram_tensor("wq", (E, E), BF16, kind="ExternalInput")      # [E, H*D], pre-scaled 1/sqrt(D)
    wk = nc.dram_tensor("wk", (E, E), BF16, kind="ExternalInput")
    wv = nc.dram_tensor("wv", (E, E), BF16, kind="ExternalInput")
    wo = nc.dram_tensor("wo", (E, E), BF16, kind="ExternalInput")
    bq = nc.dram_tensor("bq", (P, MT_Q), F32, kind="ExternalInput")   # per-partition bias per M-tile
    bk = nc.dram_tensor("bk", (P, MT_Q), F32, kind="ExternalInput")
    bo2 = nc.dram_tensor("bo2", (P, EC), F32, kind="ExternalInput")   # (bv@Wo + bo) per e_out partition
    sel2 = nc.dram_tensor("sel2", (1, D), BF16, kind="ExternalInput")  # ones row for K=1 broadcast
    out = nc.dram_tensor("out", (E, QB), F32, kind="ExternalOutput")  # y^T

    with tile.TileContext(nc) as tc:
        with ExitStack() as ctx:
            _emit(ctx, tc, nc, xq, xk, xv, wq, wk, wv, wo, bq, bk, bo2, sel2, out)
    nc.compile()
    return nc


def _emit(ctx, tc, nc, xq, xk, xv, wq, wk, wv, wo, bq, bk, bo2, sel2, out):
    # ---- pools ----
    persist = ctx.enter_context(tc.tile_pool(name="persist", bufs=1))
    # big weight slots [128, 6, 768] reused wq -> wk -> wv -> wo
    wpool = ctx.enter_context(tc.tile_pool(name="wpool", bufs=2))
    xpool = ctx.enter_context(tc.tile_pool(name="xpool", bufs=2))
    xvpool = ctx.enter_context(tc.tile_pool(name="xvpool", bufs=3))
    epool = ctx.enter_context(tc.tile_pool(name="epool", bufs=20))
    dpool = ctx.enter_context(tc.tile_pool(name="dpool", bufs=2))
    outpool = ctx.enter_context(tc.tile_pool(name="outpool", bufs=6))
    # PSUM budget 8 banks/partition: psA 2 + psB 2 + psC 4
    psA = ctx.enter_context(tc.tile_pool(name="psA", bufs=2, space="PSUM"))   # [128,512] proj qk + PV out + O-proj
    psB = ctx.enter_context(tc.tile_pool(name="psB", bufs=2, space="PSUM"))   # [128,512] v chains / bc / O-proj
    psC = ctx.enter_context(tc.tile_pool(name="psC", bufs=2, space="PSUM"))   # [128,2,512] scores

    # ---- persistent SBUF tensors ----
    qT = persist.tile([P, MT_Q, QB], BF16)       # q^T [768, QB]
    kT = persist.tile([P, MT_Q, S], BF16)        # k^T [768, S]
    v_sb = persist.tile([P, KT, H, D + 1], BF16)  # v + ones column per head
    o_all = persist.tile([P, H // 2, QB], BF16)   # normalized o^T, head pairs in partition halves
    o_raw = persist.tile([P, H // 2, QB], F32)    # unnormalized o^T, pair-stacked
    bq_sb = persist.tile([P, MT_Q], F32)
    bk_sb = persist.tile([P, MT_Q], F32)
    bo2_sb = persist.tile([P, EC], F32)
    sel_sb = persist.tile([1, D], BF16)

    # first-needed DMAs first; constants go on the scalar HWDGE queue
    wq_t = wpool.tile([P, EC, E], BF16, tag="w18")
    xq_t = xpool.tile([P, EC, QB], BF16, tag="xs")
    qs_ = [nc.sync, nc.gpsimd, nc.scalar]
    for ec in range(EC):
        qs_[ec % 3].dma_start(wq_t[:, ec, :], wq[ec * P:(ec + 1) * P, :])
        qs_[(ec + 1) % 3].dma_start(xq_t[:, ec, :], xq[ec * P:(ec + 1) * P, :])
    nc.scalar.dma_start(bq_sb[:], bq[:])
    nc.scalar.dma_start(bk_sb[:], bk[:])
    nc.scalar.dma_start(bo2_sb[:], bo2[:])
    nc.scalar.dma_start(sel_sb[:], sel2[:])

    # ones columns for denominator (written once; v-proj copies don't touch col D)
    nc.vector.memset(v_sb[:, :, :, D], 1.0)

    # ---- q^T projection ----
    for mt in range(MT_Q):
        ps = psA.tile([P, 512], F32, tag="psA")
        for ec in range(EC):
            nc.tensor.matmul(ps[:], wq_t[:, ec, mt * P:(mt + 1) * P], xq_t[:, ec, :],
                             start=(ec == 0), stop=(ec == EC - 1))
        nc.scalar.add(qT[:, mt, :], ps[:], bq_sb[:, mt:mt + 1])

    # ---- k^T projection, woven with hp0's scores+exp so ScalarE starts early ----
    def scores_exp(hp, kt):
        st = psC.tile([P, 2, 512], F32, tag="psC")
        for i in range(2):
            po = D * i      # partition offset of this head's d-rows
            nc.tensor.matmul(st[:, i, :],
                             kT[po:po + D, hp, kt * P:(kt + 1) * P],
                             qT[po:po + D, hp, :],
                             start=True, stop=True)
        ex = epool.tile([P, 2, 512], BF16, tag="ex", name=f"ex{hp}_{kt}")
        nc.scalar.activation(ex[:, :, :], st[:, :, :], mybir.ActivationFunctionType.Exp)
        return ex

    ex0 = {}
    wk_t = wpool.tile([P, EC, E], BF16, tag="w18")
    for ec in range(EC):
        qs_[ec % 3].dma_start(wk_t[:, ec, :], wk[ec * P:(ec + 1) * P, :])
    def v_proj(kt):
        # two independent 384-wide chains (heads 0-5 / 6-11) so the per-kt
        # psum->sbuf copy doesn't serialize the whole v pipeline
        xv_t = xvpool.tile([P, EC, P], BF16, tag="xv")
        nc.gpsimd.dma_start(
            xv_t[:], xv[:, kt * P:(kt + 1) * P].rearrange("(ec p) s -> p ec s", p=P))
        for half in range(2):
            # [128,512] slot (O-proj reuses these banks later); v uses 384 cols
            psv = psB.tile([P, 512], F32, tag="psB", name=f"psv{half}")
            for ec in range(EC):
                nc.tensor.matmul(psv[:, 0:384], xv_t[:, ec, :], wv_t[:, ec, 384 * half:384 * (half + 1)],
                                 start=(ec == 0), stop=(ec == EC - 1))
            # strided copy into per-head slots (leaves ones column intact)
            nc.vector.tensor_copy(v_sb[:, kt, 6 * half:6 * (half + 1), 0:D],
                                  psv[:, 0:384].rearrange("p (h d) -> p h d", d=D))

    for n4 in range(NC4):
        xk_t = xpool.tile([P, EC, 512], BF16, tag="xs")
        nc.sync.dma_start(xk_t[:], xk[:, n4 * 512:(n4 + 1) * 512].rearrange("(ec p) s -> p ec s", p=P))
        for mt in range(MT_Q):
            ps = psA.tile([P, 512], F32, tag="psA")
            for ec in range(EC):
                nc.tensor.matmul(ps[:], wk_t[:, ec, mt * P:(mt + 1) * P], xk_t[:, ec, :],
                                 start=(ec == 0), stop=(ec == EC - 1))
            nc.scalar.add(kT[:, mt, n4 * 512:(n4 + 1) * 512], ps[:], bk_sb[:, mt:mt + 1])
        for kt in range(4 * n4, 4 * n4 + 4):
            ex0[kt] = scores_exp(0, kt)

    # ---- attention: head pairs, normalize streamed per pair ----
    # Per key tile: both heads' score matmuls are adjacent K=64 ops on
    # disjoint PE row groups (partitions 0-63 / 64-127) -> run concurrently.
    def pv(hp, kt, o_ps, ex):
        for i in range(2):
            nc.tensor.matmul(o_ps[i][0:D + 1, :],
                             v_sb[:, kt, 2 * hp + i, :],
                             ex[:, i, :],
                             start=(kt == 0), stop=(kt == KT - 1))

    wv_t = wpool.tile([P, EC, E], BF16, tag="w18")
    for ec in range(EC):
        nc.sync.dma_start(wv_t[:, ec, :], wv[ec * P:(ec + 1) * P, :])
    # hp0's PV drains inside the v loop: k-proj's psA traffic is done, so the
    # pair of accumulators can stay live across it (frees ex tiles early)
    o_ps0 = {i: psA.tile([P, 512], F32, tag="psA", name=f"o_ps{i}") for i in range(2)}
    for kt in range(KT):
        v_proj(kt)
        pv(0, kt, o_ps0, ex0[kt])

    # prefetch wo for the output projection (slot reuse after wq)
    wo_t = wpool.tile([P, EC, E], BF16, tag="w18")
    nc.sync.dma_start(wo_t[:], wo[:].rearrange("(ec p) m -> p ec m", p=P))

    def normalize(hp, o_ps):
        # stage pair-stacked o_raw (frees PSUM), reciprocal of the denominator
        # rows (fast approx via SBUF bounce), bf16 cast, K=1 broadcast matmuls
        # into psB-bank bc halves, DVE multiplies.
        dens_t = dpool.tile([1, 2, 512], F32, tag="dens", name="dens")
        drec_t = dpool.tile([1, 2, 512], F32, tag="drec", name="drec")
        drec_b = dpool.tile([1, 2, 512], BF16, tag="drecb", name="drecb")
        for i in range(2):
            nc.vector.tensor_copy(o_raw[D * i:D * i + D, hp, :], o_ps[i][0:D, :])
            nc.vector.tensor_copy(dens_t[0:1, i, :], o_ps[i][D:D + 1, :])
        nc.vector.reciprocal_approx_fast(drec_t[:], dens_t[:])
        nc.vector.tensor_copy(drec_b[:], drec_t[:])
        for qh in range(2):
            bc_ps = psB.tile([P, 256], F32, tag="psB", name="bc")
            for i in range(2):
                nc.tensor.matmul(bc_ps[D * i:D * i + D, :], sel_sb[0:1, 0:D],
                                 drec_b[0:1, i, 256 * qh:256 * (qh + 1)],
                                 start=True, stop=True)
            nc.vector.tensor_tensor(o_all[:, hp, 256 * qh:256 * (qh + 1)],
                                    o_raw[:, hp, 256 * qh:256 * (qh + 1)], bc_ps[:],
                                    mybir.AluOpType.mult)

    normalize(0, o_ps0)
    for hp in range(1, H // 2):
        o_ps = {i: psA.tile([P, 512], F32, tag="psA", name=f"o_ps{i}") for i in range(2)}
        for kt in range(KT):
            ex = scores_exp(hp, kt)
            pv(hp, kt, o_ps, ex)
        normalize(hp, o_ps)

    # ---- output projection: y^T[e_out, q] per e-chunk, K=128 per head pair ----
    # hp5 contracts last so hp0-4 matmuls overlap hp5's normalize chain;
    # bf16 output halves the final write-out DMA (error ~0.2% << gate)
    # alternate psA/psB banks so two e-chunk chains pipeline concurrently
    for ec in range(EC):
        pool, tag = (psA, "psA") if ec % 2 == 0 else (psB, "psB")
        po = pool.tile([P, 512], F32, tag=tag, name="psO")
        for hp in range(H // 2):
            nc.tensor.matmul(po[:], wo_t[:, hp, ec * P:(ec + 1) * P], o_all[:, hp, :],
                             start=(hp == 0), stop=(hp == H // 2 - 1))
        out_sb = outpool.tile([P, 512], F32, tag="outsb")
        nc.vector.tensor_scalar_add(out_sb[:], po[:], bo2_sb[:, ec:ec + 1])
        qs_[ec % 3].dma_start(out[ec * P:(ec + 1) * P, :], out_sb[:])


_NC_CACHE = None


def _get_nc():
    global _NC_CACHE
    if _NC_CACHE is None:
        _NC_CACHE = build_nc()
    return _NC_CACHE


def make_in_maps(query, key_, value, Wq, bq, Wk, bk, Wv, bv, Wo, bo):
    """Host-side sharding + layout prep. Returns list of 8 input dicts."""
    query = np.asarray(query, dtype=np.float32)
    key_ = np.asarray(key_, dtype=np.float32)
    value = np.asarray(value, dtype=np.float32)
    scale = 1.0 / np.sqrt(np.float32(D))

    import ml_dtypes
    BF = ml_dtypes.bfloat16
    wq_f = (np.ascontiguousarray(np.transpose(np.asarray(Wq, np.float32), (1, 0, 2)).reshape(E, E)) * scale).astype(BF)
    wk_f = np.ascontiguousarray(np.transpose(np.asarray(Wk, np.float32), (1, 0, 2)).reshape(E, E)).astype(BF)
    wv_f = np.ascontiguousarray(np.transpose(np.asarray(Wv, np.float32), (1, 0, 2)).reshape(E, E)).astype(BF)
    wo_f = np.ascontiguousarray(np.asarray(Wo, np.float32)).astype(BF)

    bq_f = (np.asarray(bq, np.float32).reshape(E) * scale).reshape(MT_Q, P).T.copy()
    bk_f = np.asarray(bk, np.float32).reshape(E).reshape(MT_Q, P).T.copy()
    bv_f = np.asarray(bv, np.float32).reshape(E)
    wo_f32 = wo_f.astype(np.float32)
    bo2_f = (bv_f @ wo_f32 + np.asarray(bo, np.float32)).reshape(EC, P).T.copy()

    xk_t = [np.ascontiguousarray(key_[b].T).astype(BF) for b in range(B)]
    xv_t = [np.ascontiguousarray(value[b].T).astype(BF) for b in range(B)]

    sel_np = np.ones((1, D), dtype=np.float32).astype(BF)

    in_maps = []
    for core in range(NCORES):
        b = core // (NCORES // B)
        qc = core % (NCORES // B)
        xq_t = np.ascontiguousarray(query[b, qc * QB:(qc + 1) * QB, :].T).astype(BF)
        in_maps.append({
            "xq": xq_t, "xk": xk_t[b], "xv": xv_t[b],
            "wq": wq_f, "wk": wk_f, "wv": wv_f, "wo": wo_f,
            "bq": bq_f, "bk": bk_f, "bo2": bo2_f, "sel2": sel_np,
        })
    return in_maps


def assemble(results):
    outp = np.empty((B, S, E), dtype=np.float32)
    for core in range(NCORES):
        b = core // (NCORES // B)
        qc = core % (NCORES // B)
        outp[b, qc * QB:(qc + 1) * QB, :] = results[core]["out"].T.astype(np.float32)
    return outp


def kernel(query, key_, value, Wq, bq, Wk, bk, Wv, bv, Wo, bo):
    nc = _get_nc()
    in_maps = make_in_maps(query, key_, value, Wq, bq, Wk, bk, Wv, bv, Wo, bo)
    res = run_bass_kernel_spmd(nc, in_maps, core_ids=list(range(NCORES)))
    return assemble(res.results)
